# revision 33
# baseline (speedup 1.0000x reference)
"""GQA attention (B=2,T=2048,E=2048,H=16,Hkv=8,D=128) with RoPE + causal mask,
sharded over 8 NeuronCores: core = (batch b, head-group g) with q-heads
{2g,2g+1,2g+8,2g+9} and kv-heads {2g,2g+1}. Each core computes its 4 heads'
attention for the full sequence plus its partial output projection; the host
sums the 4 partials per batch.

Device layout notes:
 - All matmul operands are bf16 (cast on host / on device); PSUM accumulation
   stays fp32. Input/output DMA is halved and every weight loads as a single
   batched DMA from a partition-major host layout.
 - Q/K are produced directly in [d, t] (transposed) layout with even/odd
   RoPE deinterleave folded into host-permuted weight columns; RoPE rotation
   is 4 DVE ops per [128,512] tile using cross-base-partition reads, and the
   K->Q->V projection phasing keeps it off the PE critical path.
 - Scores are computed as S^T [kv,q] so no P transposes are needed. Causal
   chunks are trimmed on the moving (q) axis to the first visible column.
 - Softmax weights (exp output), V, and the mask are bf16: DVE runs at 2x
   rate accumulating exp tiles into an SBUF accumulator per head; a single
   ones-column matmul per head reduces it, and a K=1 broadcast matmul + DVE
   multiply normalizes the [D,q] attention output.
 - Score and AV matmuls are software-pipelined across head boundaries (AV
   lags scores by 2 pair-tiles) so the PE never waits on exp latency.
"""
import sys
if "/opt/trn_rl_repo" not in sys.path:
    sys.path.insert(0, "/opt/trn_rl_repo")
from contextlib import ExitStack

import numpy as np
from ml_dtypes import bfloat16 as np_bf16

import concourse.tile as tile
from concourse import bacc, mybir
from concourse.bass_utils import run_bass_kernel_spmd

F32 = mybir.dt.float32
F32R = mybir.dt.float32r
BF16 = mybir.dt.bfloat16
EXP = mybir.ActivationFunctionType.Exp
COPY = mybir.ActivationFunctionType.Copy

B, T, E = 2, 2048, 2048
H, H_KV, D = 16, 8, 128
TG = 512                 # q-group / moving-dim tile
NTG = T // TG            # 4
NEC = E // 128           # 16 contraction chunks for projections
SCALE = float(D) ** -0.5

_cache: dict = {}


def _build_program(cfg):
    """cfg: tuple per qg of tuple of (chunk_index, mask_tile_idx or -1,
    s0 trim start, mask-mul start, mask-mul end)."""
    nmt = max(1, 1 + max((mi for qgc in cfg for _, mi, _, _, _ in qgc), default=-1))
    nc = bacc.Bacc("TRN2", target_bir_lowering=False, debug=False, num_devices=8)

    # partition-major host layouts: one flat DMA per tensor
    xT = nc.dram_tensor("xT", [128, NEC, T], BF16, kind="ExternalInput").ap()
    wqa = nc.dram_tensor("wqa", [128, NEC * 4 * D], BF16, kind="ExternalInput").ap()
    wka = nc.dram_tensor("wka", [128, NEC * 2 * D], BF16, kind="ExternalInput").ap()
    wvg = nc.dram_tensor("wvg", [128, NEC * 2 * D], BF16, kind="ExternalInput").ap()
    wog = nc.dram_tensor("wog", [128, 4 * E], BF16, kind="ExternalInput").ap()
    c2d = nc.dram_tensor("c2", [128, T], F32, kind="ExternalInput").ap()
    s2d = nc.dram_tensor("s2", [128, T], F32, kind="ExternalInput").ap()
    onesd = nc.dram_tensor("ones", [128, 128], F32R, kind="ExternalInput").ap()
    onesbd = nc.dram_tensor("onesb", [128, 1], BF16, kind="ExternalInput").ap()
    mtd = nc.dram_tensor("mt", [128, nmt * TG], BF16, kind="ExternalInput").ap()
    out = nc.dram_tensor("o", [T, E], BF16, kind="ExternalOutput").ap()
    causal_compat = all(
        (max(c for c, _, _, _, _ in qgc) * 128) // TG <= qg
        for qg, qgc in enumerate(cfg) if qgc)
    mask_resident = nmt <= 4
    qscr = None
    if not causal_compat:
        qscr = nc.dram_tensor("qscr", [NTG, 4, 128, TG], BF16, kind="Internal").ap()

    fastpath = causal_compat and mask_resident
    with tile.TileContext(nc) as tc, ExitStack() as ctx:
        cp = ctx.enter_context(tc.tile_pool(name="const", bufs=1))
        pairp = ctx.enter_context(tc.tile_pool(name="pairp", bufs=2, space="PSUM"))
        singp = ctx.enter_context(tc.tile_pool(name="singp", bufs=4, space="PSUM"))
        xp = ctx.enter_context(tc.tile_pool(name="xp", bufs=2))
        m2p = ctx.enter_context(tc.tile_pool(name="m2p", bufs=2))
        ptp = ctx.enter_context(tc.tile_pool(name="ptp", bufs=4 if fastpath else 3))
        atp = ctx.enter_context(tc.tile_pool(name="atp", bufs=1))
        accp = ctx.enter_context(tc.tile_pool(name="accp", bufs=3))
        obp = ctx.enter_context(tc.tile_pool(name="obp", bufs=3 if fastpath else 2))
        rcp = ctx.enter_context(tc.tile_pool(name="rcp", bufs=2))

        # ---- resident constants (single batched DMA per tensor) ----
        wq_b = cp.tile([128, NEC * 4 * D], BF16, tag="wq", name="wq")
        wk_b = cp.tile([128, NEC * 2 * D], BF16, tag="wk", name="wk")
        wv_b = cp.tile([128, NEC * 2 * D], BF16, tag="wv", name="wv")
        wo_b = cp.tile([128, 4 * E], BF16, tag="wo", name="wo")
        c2_b = cp.tile([128, T], F32, tag="c2", name="c2")
        s2_b = cp.tile([128, T], F32, tag="s2", name="s2")
        ones_colb = cp.tile([128, 1], BF16, tag="ones_colb", name="ones_colb")
        nc.sync.dma_start(ones_colb[:], onesbd[:, :])
        ones_row = cp.tile([1, 128], F32R, tag="ones_row", name="ones_row")
        nc.sync.dma_start(ones_row[:], onesd[0:1, :])
        mt_b = (cp.tile([128, nmt * TG], BF16, tag="mt", name="mt")
                if mask_resident else None)
        msp = None if mask_resident else ctx.enter_context(tc.tile_pool(name="msp", bufs=2))
        # K^T (rotated) and V persist for the whole sequence
        krot = [cp.tile([128, T], BF16, tag=f"kr{lk}", name=f"kr{lk}") for lk in range(2)]
        vres = [cp.tile([128, 2 * D], BF16, tag=f"v{t}", name=f"v{t}") for t in range(T // 128)]

        def prefetch_x(tg):
            tgs = slice(tg * TG, (tg + 1) * TG)
            xtb = xp.tile([128, NEC * TG], BF16, tag="xt", name=f"xt{tg}")
            for g in range(4):
                if tg == 0:
                    nc.sync.dma_start(wk_b[:, g * 4 * 2 * D:(g + 1) * 4 * 2 * D],
                                      wka[:, g * 4 * 2 * D:(g + 1) * 4 * 2 * D])
                nc.sync.dma_start(xtb[:, g * 4 * TG:(g + 1) * 4 * TG],
                                  xT[:, 4 * g:4 * g + 4, tgs])
                if tg == 0:
                    nc.sync.dma_start(wq_b[:, g * 4 * 4 * D:(g + 1) * 4 * 4 * D],
                                      wqa[:, g * 4 * 4 * D:(g + 1) * 4 * 4 * D])
            if tg == 0:
                nc.sync.dma_start(c2_b[:], c2d[:, :])
                nc.sync.dma_start(s2_b[:], s2d[:, :])
                nc.sync.dma_start(wv_b[:], wvg[:, :])
            return xtb

        def emit_proj(tg, xtb):
            """Projections for t-group tg in three phases (K -> Q -> V) so the
            DVE RoPE chain for K/Q overlaps the next phase's PE matmuls."""
            tgs = slice(tg * TG, (tg + 1) * TG)

            def xte(e):
                return xtb[:, e * TG:(e + 1) * TG]

            def rope(dst, src):
                m2 = m2p.tile([128, TG], F32, tag="m2")
                nc.vector.tensor_mul(m2[:], src, s2_b[:, tgs])
                nc.vector.tensor_mul(src, src, c2_b[:, tgs])
                with nc.allow_low_precision(reason="bf16 rotated q/k for scores"):
                    nc.vector.tensor_sub(dst[0:64, :], src[0:64, :], m2[64:128, :])
                    nc.vector.tensor_add(dst[64:128, :], src[64:128, :], m2[0:64, :])

            # --- phase K ---
            kp = [singp.tile([128, TG], F32, tag="sg", name=f"kp{lk}_{tg}") for lk in range(2)]
            for e in range(NEC):
                for lk in range(2):
                    nc.tensor.matmul(kp[lk][:], wk_b[:, e * 2 * D + lk * D:e * 2 * D + (lk + 1) * D],
                                     xte(e), start=(e == 0), stop=(e == NEC - 1))
            rope(krot[0][:, tgs], kp[0][:])
            rope(krot[1][:, tgs], kp[1][:])

            # --- phase Q ---
            qp2 = [pairp.tile([128, 2 * TG], F32, tag="pp", name=f"qp{i}_{tg}") for i in range(2)]
            qp = [qp2[lh // 2][:, (lh % 2) * TG:(lh % 2 + 1) * TG] for lh in range(4)]
            for e in range(NEC):
                for lh in range(4):
                    nc.tensor.matmul(qp[lh], wq_b[:, e * 4 * D + lh * D:e * 4 * D + (lh + 1) * D],
                                     xte(e), start=(e == 0), stop=(e == NEC - 1))
            qrot = [atp.tile([128, TG], BF16, tag=f"qr{lh}", name=f"qr{lh}_{tg}")
                    for lh in range(4)]
            for lh in range(4):
                rope(qrot[lh][:], qp[lh])

            # --- phase V ---
            vp = [singp.tile([128, TG], F32, tag="sg", name=f"vp{tsp}_{tg}") for tsp in range(2)]
            for e in range(NEC):
                for tsp in range(2):
                    for k in range(2):
                        ts = tsp * 2 + k
                        nc.tensor.matmul(vp[tsp][:, k * 256:(k + 1) * 256],
                                         xte(e)[:, ts * 128:(ts + 1) * 128],
                                         wv_b[:, e * 2 * D:(e + 1) * 2 * D],
                                         start=(e == 0 and k == 0), stop=(e == NEC - 1),
                                         skip_group_check=True)
            for tsp in range(2):
                for k in range(2):
                    ts = tsp * 2 + k
                    nc.scalar.copy(vres[tg * 4 + ts][:], vp[tsp][:, k * 256:(k + 1) * 256])
            if qscr is not None:
                for lh in range(4):
                    nc.sync.dma_start(qscr[tg, lh, :, :], qrot[lh][:])
            return qrot

        def emit_attention(qg, qrot):
            """Attention for q-group qg. Returns the 4 normalized attnT tiles.
            Score/exp emission runs 2 pair-tiles ahead of the AV matmuls,
            crossing head boundaries, so PE never waits on exp latency."""
            chunks = cfg[qg]
            nck = len(chunks)
            at_t = [atp.tile([128, TG], BF16, tag=f"at{lh}", name=f"at{qg}_{lh}")
                    for lh in range(4)]
            if nck == 0:
                for lh in range(4):
                    nc.vector.memset(at_t[lh][:], 0.0)
                return at_t

            pairs = [chunks[i:i + 2] for i in range(0, nck, 2)]
            npairs = len(pairs)
            units = [(lh, pi) for lh in range(4) for pi in range(npairs)]
            odt, acct, pts = {}, {}, {}
            norm_pending = []      # head indices whose AV is fully emitted
            avdone = {}            # head -> unit index when its last AV was emitted
            pending = []

            def emit_av(lh, pi):
                pt, pair = pts.pop((lh, pi))
                lk = lh % 2
                for k, (c, mi, s0, m0, m1) in enumerate(pair):
                    idx = 2 * pi + k
                    nc.tensor.matmul(odt[lh][:, s0:], vres[c][:, lk * D:(lk + 1) * D],
                                     pt[:, k * TG + s0:(k + 1) * TG],
                                     start=(idx == 0), stop=(idx == nck - 1),
                                     skip_group_check=True)
                if pi == npairs - 1:
                    norm_pending.append(lh)

            def finish_normalize():
                lh = norm_pending.pop(0)
                ddp = singp.tile([1, TG], F32, tag="sg", name=f"dd{qg}_{lh}")
                nc.tensor.matmul(ddp[:], ones_colb[:], acct[lh][:], start=True, stop=True)
                den = rcp.tile([1, TG], F32R, tag="den", name=f"den{qg}_{lh}")
                nc.scalar.activation(den[:], ddp[:], COPY, bias=1e-30)
                bt = singp.tile([128, TG], F32, tag="sg", name=f"bt{qg}_{lh}")
                nc.tensor.matmul(bt[:], ones_row[:], den[:], start=True, stop=True)
                rec = rcp.tile([128, TG], F32, tag="rec", name=f"rec{qg}_{lh}")
                nc.vector.reciprocal_approx_fast(rec[:], bt[:])
                with nc.allow_low_precision(reason="bf16 attn output for wo matmul"):
                    nc.vector.tensor_mul(at_t[lh][:], odt[lh][:], rec[:])

            for ui, (lh, pi) in enumerate(units):
                if norm_pending and ui > avdone.get(norm_pending[0], 1 << 30):
                    finish_normalize()   # >=1 unit of slack after the last AV
                if pi == 0:
                    odt[lh] = singp.tile([128, TG], F32, tag="sg", name=f"od{qg}_{lh}")
                    acct[lh] = accp.tile([128, TG], BF16, tag="acc", name=f"acc{qg}_{lh}")
                lk = lh % 2
                pair = pairs[pi]
                sp = pairp.tile([128, 2 * TG], F32, tag="pp", name=f"sp{qg}_{lh}_{pi}")
                for k, (c, mi, s0, m0, m1) in enumerate(pair):
                    nc.tensor.matmul(sp[:, k * TG + s0:(k + 1) * TG],
                                     krot[lk][:, c * 128:(c + 1) * 128],
                                     qrot[lh][:, s0:], start=True, stop=True)
                pt = ptp.tile([128, 2 * TG], BF16, tag="pt", name=f"pt{qg}_{lh}_{pi}")
                lo = pair[0][2]
                hi = (len(pair) - 1) * TG + TG
                with nc.allow_low_precision(reason="bf16 softmax weights"):
                    nc.scalar.activation(pt[:, lo:hi], sp[:, lo:hi], EXP, scale=SCALE)
                for k, (c, mi, s0, m0, m1) in enumerate(pair):
                    if mi >= 0:
                        if mask_resident:
                            mtile = mt_b[:, mi * TG + m0:mi * TG + m1]
                        else:
                            mst = msp.tile([128, TG], BF16, tag="ms",
                                           name=f"ms{qg}_{lh}_{pi}")
                            nc.sync.dma_start(mst[:], mtd[:, mi * TG:(mi + 1) * TG])
                            mtile = mst[:, m0:m1]
                        nc.vector.tensor_mul(pt[:, k * TG + m0:k * TG + m1],
                                             pt[:, k * TG + m0:k * TG + m1],
                                             mtile)
                with nc.allow_low_precision(reason="bf16 denominator accumulator"):
                    for k, (c, mi, s0, m0, m1) in enumerate(pair):
                        idx = 2 * pi + k
                        if idx == 0:
                            nc.vector.tensor_copy(acct[lh][:], pt[:, 0:TG])
                        else:
                            nc.vector.tensor_add(acct[lh][:, s0:], acct[lh][:, s0:],
                                                 pt[:, k * TG + s0:(k + 1) * TG])
                pts[(lh, pi)] = (pt, pair)
                pending.append((lh, pi))
                while len(pending) > 2:
                    u = pending.pop(0)
                    emit_av(*u)
                    if u[1] == npairs - 1:
                        avdone[u[0]] = ui
            while pending:
                u = pending.pop(0)
                emit_av(*u)
            while norm_pending:
                finish_normalize()
            return at_t

        def emit_wo(qg, at_t):
            for ts in range(4):
                ob = obp.tile([128, E], BF16, tag="ob", name=f"ob{qg}_{ts}")
                for eb in range(NTG):
                    wps = singp.tile([128, TG], F32, tag="sg", name=f"w{qg}_{ts}_{eb}")
                    for lh in range(4):
                        nc.tensor.matmul(wps[:], at_t[lh][:, ts * 128:(ts + 1) * 128],
                                         wo_b[:, lh * E + eb * TG:lh * E + (eb + 1) * TG],
                                         start=(lh == 0), stop=(lh == 3))
                    if eb % 2 == 0:
                        nc.scalar.copy(ob[:, eb * TG:(eb + 1) * TG], wps[:])
                    else:
                        with nc.allow_low_precision(reason="bf16 output tile"):
                            nc.vector.tensor_copy(ob[:, eb * TG:(eb + 1) * TG], wps[:])
                nc.sync.dma_start(
                    out[qg * TG + ts * 128:qg * TG + (ts + 1) * 128, :], ob[:])

        if causal_compat:
            # schedule: proj0 attn0 proj1 wo0 attn1 proj2 wo1 attn2 proj3 wo2 attn3 wo3
            qrot = emit_proj(0, prefetch_x(0))
            if mt_b is not None:
                nc.sync.dma_start(mt_b[:], mtd[:, :])
            nc.sync.dma_start(wo_b[:], wog[:, :])
            xtb_next = prefetch_x(1)
            for tg in range(NTG):
                at_t = emit_attention(tg, qrot)
                if tg + 1 < NTG:
                    qrot = emit_proj(tg + 1, xtb_next)
                    if tg + 2 < NTG:
                        xtb_next = prefetch_x(tg + 2)
                emit_wo(tg, at_t)
        else:
            # general masks: project everything (Q^T spilled to DRAM), then attend
            if mt_b is not None:
                nc.sync.dma_start(mt_b[:], mtd[:, :])
            nc.sync.dma_start(wo_b[:], wog[:, :])
            for tg in range(NTG):
                emit_proj(tg, prefetch_x(tg))
            for qg in range(NTG):
                qrot = [atp.tile([128, TG], BF16, tag=f"qr{lh}", name=f"ql{qg}_{lh}")
                        for lh in range(4)]
                for lh in range(4):
                    nc.sync.dma_start(qrot[lh][:], qscr[qg, lh, :, :])
                at_t = emit_attention(qg, qrot)
                emit_wo(qg, at_t)

    nc.compile()
    return nc


def _host_prep(x, mask, wq, wk, wv, wo):
    m2dm = np.asarray(mask).reshape(T, T) != 0
    cfg = []
    mask_tiles = []
    mask_key = {}
    for qg in range(NTG):
        qs = slice(qg * TG, (qg + 1) * TG)
        lst = []
        for c in range(T // 128):
            sub = m2dm[qs, c * 128:(c + 1) * 128]   # [512 q, 128 kv]
            if not sub.any():
                continue
            vis_q = sub.any(1)
            full_q = sub.all(1)
            q0 = int(np.argmax(vis_q))
            s0 = q0
            nf = ~full_q
            nf[:s0] = False
            if nf.any():
                m0 = int(np.argmax(nf))
                m1 = int(len(nf) - np.argmax(nf[::-1]))
                tilea = np.ascontiguousarray(sub.T).astype(np_bf16)
                key = tilea.tobytes()
                if key not in mask_key:
                    mask_key[key] = len(mask_tiles)
                    mask_tiles.append(tilea)
                mi = mask_key[key]
            else:
                mi, m0, m1 = -1, 0, 0
            lst.append((c, mi, s0, m0, m1))
        cfg.append(tuple(lst))
    cfg = tuple(cfg)
    if mask_tiles:
        mt = np.ascontiguousarray(np.concatenate(mask_tiles, 1))   # [128, nmt*TG]
    else:
        mt = np.zeros((128, TG), np_bf16)

    inv = 1.0 / (10000.0 ** (np.arange(64, dtype=np.float64) / 64))
    ang = np.arange(T, dtype=np.float64)[:, None] * inv[None, :]
    c64 = np.cos(ang).astype(np.float32).T
    s64 = np.sin(ang).astype(np.float32).T
    c2 = np.ascontiguousarray(np.concatenate([c64, c64], 0))
    s2 = np.ascontiguousarray(np.concatenate([s64, s64], 0))
    ones = np.ones((128, 128), np.float32)
    onesb = np.ones((128, 1), np_bf16)

    def pmaj(a, nchunk):
        """[nchunk*128, W] -> partition-major [128, nchunk*W]."""
        w = a.shape[1]
        return np.ascontiguousarray(
            a.reshape(nchunk, 128, w).transpose(1, 0, 2).reshape(128, nchunk * w))

    in_maps = []
    for b in range(B):
        xTb = pmaj(np.ascontiguousarray(x[b].T), NEC).reshape(128, NEC, T).astype(np_bf16)
        for g in range(4):
            heads = [2 * g, 2 * g + 1, 2 * g + 8, 2 * g + 9]
            kvh = [2 * g, 2 * g + 1]
            wq_a = np.empty((E, 4 * D), np.float32)
            for lh, h in enumerate(heads):
                wq_a[:, lh * D:lh * D + 64] = wq[:, h * D:(h + 1) * D:2]
                wq_a[:, lh * D + 64:(lh + 1) * D] = wq[:, h * D + 1:(h + 1) * D:2]
            wk_a = np.empty((E, 2 * D), np.float32)
            for lk, h in enumerate(kvh):
                wk_a[:, lk * D:lk * D + 64] = wk[:, h * D:(h + 1) * D:2]
                wk_a[:, lk * D + 64:(lk + 1) * D] = wk[:, h * D + 1:(h + 1) * D:2]
            wv_g = np.ascontiguousarray(wv[:, kvh[0] * D:(kvh[0] + 2) * D])
            wo_g = np.concatenate([wo[h * D:(h + 1) * D] for h in heads], 0)
            in_maps.append({
                "xT": xTb,
                "wqa": pmaj(wq_a, NEC).astype(np_bf16),
                "wka": pmaj(wk_a, NEC).astype(np_bf16),
                "wvg": pmaj(wv_g, NEC).astype(np_bf16),
                "wog": pmaj(wo_g, 4).astype(np_bf16),
                "c2": c2, "s2": s2, "ones": ones, "onesb": onesb, "mt": mt,
            })
    return cfg, in_maps


def kernel(x, mask, wq, wk, wv, wo, _profile=None):
    x = np.ascontiguousarray(np.asarray(x, dtype=np.float32))
    wq = np.asarray(wq, dtype=np.float32)
    wk = np.asarray(wk, dtype=np.float32)
    wv = np.asarray(wv, dtype=np.float32)
    wo = np.asarray(wo, dtype=np.float32)
    cfg, in_maps = _host_prep(x, mask, wq, wk, wv, wo)
    if cfg not in _cache:
        _cache[cfg] = _build_program(cfg)
    nc = _cache[cfg]
    kwargs = dict(_profile) if _profile else {}
    res = run_bass_kernel_spmd(nc, in_maps, core_ids=list(range(8)), **kwargs)
    if _profile is not None and isinstance(_profile, dict):
        _profile["result"] = res
    outp = np.zeros((B, T, E), np.float32)
    for b in range(B):
        for g in range(4):
            outp[b] += res.results[b * 4 + g]["o"].astype(np.float32)
    return outp


# revision 40
# speedup vs baseline: 1.2576x; 1.2576x over previous
"""GQA attention (B=2,T=2048,E=2048,H=16,Hkv=8,D=128) with RoPE + causal mask,
sharded over 8 NeuronCores: core = (batch b, head-group g) with q-heads
{2g,2g+1,2g+8,2g+9} and kv-heads {2g,2g+1}. Each core computes its 4 heads'
attention for the full sequence plus its partial output projection; the host
sums the 4 partials per batch.

Device layout notes:
 - All matmul operands are bf16 (cast on host / on device); PSUM accumulation
   stays fp32. Input/output DMA is halved and every weight loads as a single
   batched DMA from a partition-major host layout.
 - Q/K are produced directly in [d, t] (transposed) layout with even/odd
   RoPE deinterleave folded into host-permuted weight columns; RoPE rotation
   is 4 DVE ops per [128,512] tile using cross-base-partition reads, and the
   K->Q->V projection phasing keeps it off the PE critical path.
 - Scores are computed as S^T [kv,q] so no P transposes are needed. Causal
   chunks are trimmed on the moving (q) axis to the first visible column.
 - Softmax weights (exp output), V, and the mask are bf16: DVE runs at 2x
   rate accumulating exp tiles into an SBUF accumulator per head; a single
   ones-column matmul per head reduces it, and a K=1 broadcast matmul + DVE
   multiply normalizes the [D,q] attention output.
 - Score and AV matmuls are software-pipelined across head boundaries (AV
   lags scores by 2 pair-tiles) so the PE never waits on exp latency.
"""
import sys
if "/opt/trn_rl_repo" not in sys.path:
    sys.path.insert(0, "/opt/trn_rl_repo")
from contextlib import ExitStack

import numpy as np
from ml_dtypes import bfloat16 as np_bf16

import concourse.tile as tile
from concourse import bacc, mybir
from concourse.bass_utils import run_bass_kernel_spmd

F32 = mybir.dt.float32
F32R = mybir.dt.float32r
BF16 = mybir.dt.bfloat16
EXP = mybir.ActivationFunctionType.Exp
COPY = mybir.ActivationFunctionType.Copy

B, T, E = 2, 2048, 2048
H, H_KV, D = 16, 8, 128
TG = 512                 # q-group / moving-dim tile
NTG = T // TG            # 4
NEC = E // 128           # 16 contraction chunks for projections
SCALE = float(D) ** -0.5

_cache: dict = {}


def _build_program(cfg, rows_ok):
    """cfg: tuple per qg of tuple of (chunk_index, mask_tile_idx or -1,
    s0 trim start, mask-mul start, mask-mul end). rows_ok: every query row
    has at least one visible key (=> denominators strictly positive)."""
    nmt = max(1, 1 + max((mi for qgc in cfg for _, mi, _, _, _ in qgc), default=-1))
    nc = bacc.Bacc("TRN2", target_bir_lowering=False, debug=False, num_devices=8)

    # partition-major host layouts: one flat DMA per tensor
    xT = nc.dram_tensor("xT", [128, NEC, T], BF16, kind="ExternalInput").ap()
    wqa = nc.dram_tensor("wqa", [128, NEC * 4 * D], BF16, kind="ExternalInput").ap()
    wka = nc.dram_tensor("wka", [128, NEC * 2 * D], BF16, kind="ExternalInput").ap()
    wvg = nc.dram_tensor("wvg", [128, NEC * 2 * D], BF16, kind="ExternalInput").ap()
    wog = nc.dram_tensor("wog", [128, 4 * E], BF16, kind="ExternalInput").ap()
    c2d = nc.dram_tensor("c2", [128, T], F32, kind="ExternalInput").ap()
    s2d = nc.dram_tensor("s2", [128, T], F32, kind="ExternalInput").ap()
    onesd = nc.dram_tensor("ones", [128, 128], F32R, kind="ExternalInput").ap()
    onesbd = nc.dram_tensor("onesb", [128, 128], BF16, kind="ExternalInput").ap()
    mtd = nc.dram_tensor("mt", [128, nmt * TG], BF16, kind="ExternalInput").ap()
    out = nc.dram_tensor("o", [T, E], BF16, kind="ExternalOutput").ap()
    causal_compat = all(
        (max(c for c, _, _, _, _ in qgc) * 128) // TG <= qg
        for qg, qgc in enumerate(cfg) if qgc)
    mask_resident = nmt <= 4
    qscr = None
    if not causal_compat:
        qscr = nc.dram_tensor("qscr", [NTG, 4, 128, TG], BF16, kind="Internal").ap()

    fastpath = causal_compat and mask_resident
    with tile.TileContext(nc) as tc, ExitStack() as ctx:
        cp = ctx.enter_context(tc.tile_pool(name="const", bufs=1))
        pairp = ctx.enter_context(tc.tile_pool(name="pairp", bufs=2, space="PSUM"))
        singp = ctx.enter_context(tc.tile_pool(name="singp", bufs=4, space="PSUM"))
        xp = ctx.enter_context(tc.tile_pool(name="xp", bufs=2))
        m2p = ctx.enter_context(tc.tile_pool(name="m2p", bufs=2))
        ptp = ctx.enter_context(tc.tile_pool(name="ptp", bufs=4 if fastpath else 3))
        atp = ctx.enter_context(tc.tile_pool(name="atp", bufs=1))
        accp = ctx.enter_context(tc.tile_pool(name="accp", bufs=3))
        obp = ctx.enter_context(tc.tile_pool(name="obp", bufs=3 if fastpath else 2))
        rcp = ctx.enter_context(tc.tile_pool(name="rcp", bufs=2))

        # ---- resident constants (single batched DMA per tensor) ----
        wq_b = cp.tile([128, NEC * 4 * D], BF16, tag="wq", name="wq")
        wk_b = cp.tile([128, NEC * 2 * D], BF16, tag="wk", name="wk")
        wv_b = cp.tile([128, NEC * 2 * D], BF16, tag="wv", name="wv")
        wo_b = cp.tile([128, 4 * E], BF16, tag="wo", name="wo")
        c2_b = cp.tile([128, T], F32, tag="c2", name="c2")
        s2_b = cp.tile([128, T], F32, tag="s2", name="s2")
        ones_b = cp.tile([128, 128], BF16, tag="ones_b", name="ones_b")
        nc.sync.dma_start(ones_b[:], onesbd[:, :])
        ones_row = cp.tile([1, 128], F32R, tag="ones_row", name="ones_row")
        nc.sync.dma_start(ones_row[:], onesd[0:1, :])
        mt_b = (cp.tile([128, nmt * TG], BF16, tag="mt", name="mt")
                if mask_resident else None)
        msp = None if mask_resident else ctx.enter_context(tc.tile_pool(name="msp", bufs=2))
        # K^T (rotated) and V persist for the whole sequence
        krot = [cp.tile([128, T], BF16, tag=f"kr{lk}", name=f"kr{lk}") for lk in range(2)]
        vres = [cp.tile([128, 2 * D], BF16, tag=f"v{t}", name=f"v{t}") for t in range(T // 128)]

        def prefetch_x(tg):
            tgs = slice(tg * TG, (tg + 1) * TG)
            xtb = xp.tile([128, NEC * TG], BF16, tag="xt", name=f"xt{tg}")
            for g in range(4):
                if tg == 0:
                    nc.sync.dma_start(wk_b[:, g * 4 * 2 * D:(g + 1) * 4 * 2 * D],
                                      wka[:, g * 4 * 2 * D:(g + 1) * 4 * 2 * D])
                nc.sync.dma_start(xtb[:, g * 4 * TG:(g + 1) * 4 * TG],
                                  xT[:, 4 * g:4 * g + 4, tgs])
                if tg == 0:
                    nc.sync.dma_start(wq_b[:, g * 4 * 4 * D:(g + 1) * 4 * 4 * D],
                                      wqa[:, g * 4 * 4 * D:(g + 1) * 4 * 4 * D])
            if tg == 0:
                nc.sync.dma_start(c2_b[:], c2d[:, :])
                nc.sync.dma_start(s2_b[:], s2d[:, :])
                nc.sync.dma_start(wv_b[:], wvg[:, :])
            return xtb

        def emit_proj(tg, xtb):
            """Projections for t-group tg in three phases (K -> Q -> V) so the
            DVE RoPE chain for K/Q overlaps the next phase's PE matmuls."""
            tgs = slice(tg * TG, (tg + 1) * TG)

            def xte(e):
                return xtb[:, e * TG:(e + 1) * TG]

            def rope(dst, src):
                m2 = m2p.tile([128, TG], F32, tag="m2")
                nc.vector.tensor_mul(m2[:], src, s2_b[:, tgs])
                nc.vector.tensor_mul(src, src, c2_b[:, tgs])
                with nc.allow_low_precision(reason="bf16 rotated q/k for scores"):
                    nc.vector.tensor_sub(dst[0:64, :], src[0:64, :], m2[64:128, :])
                    nc.vector.tensor_add(dst[64:128, :], src[64:128, :], m2[0:64, :])

            # --- phase K ---
            kp = [singp.tile([128, TG], F32, tag="sg", name=f"kp{lk}_{tg}") for lk in range(2)]
            for e in range(NEC):
                for lk in range(2):
                    nc.tensor.matmul(kp[lk][:], wk_b[:, e * 2 * D + lk * D:e * 2 * D + (lk + 1) * D],
                                     xte(e), start=(e == 0), stop=(e == NEC - 1))
            rope(krot[0][:, tgs], kp[0][:])
            rope(krot[1][:, tgs], kp[1][:])

            # --- phase Q ---
            qp2 = [pairp.tile([128, 2 * TG], F32, tag="pp", name=f"qp{i}_{tg}") for i in range(2)]
            qp = [qp2[lh // 2][:, (lh % 2) * TG:(lh % 2 + 1) * TG] for lh in range(4)]
            for e in range(NEC):
                for lh in range(4):
                    nc.tensor.matmul(qp[lh], wq_b[:, e * 4 * D + lh * D:e * 4 * D + (lh + 1) * D],
                                     xte(e), start=(e == 0), stop=(e == NEC - 1))
            qrot = [atp.tile([128, TG], BF16, tag=f"qr{lh}", name=f"qr{lh}_{tg}")
                    for lh in range(4)]
            for lh in range(4):
                rope(qrot[lh][:], qp[lh])

            # --- phase V ---
            vp = [singp.tile([128, TG], F32, tag="sg", name=f"vp{tsp}_{tg}") for tsp in range(2)]
            for e in range(NEC):
                for tsp in range(2):
                    for k in range(2):
                        ts = tsp * 2 + k
                        nc.tensor.matmul(vp[tsp][:, k * 256:(k + 1) * 256],
                                         xte(e)[:, ts * 128:(ts + 1) * 128],
                                         wv_b[:, e * 2 * D:(e + 1) * 2 * D],
                                         start=(e == 0 and k == 0), stop=(e == NEC - 1),
                                         skip_group_check=True)
            for tsp in range(2):
                for k in range(2):
                    ts = tsp * 2 + k
                    nc.scalar.copy(vres[tg * 4 + ts][:], vp[tsp][:, k * 256:(k + 1) * 256])
            if qscr is not None:
                for lh in range(4):
                    nc.sync.dma_start(qscr[tg, lh, :, :], qrot[lh][:])
            return qrot

        def emit_attention(qg, qrot):
            """Attention for q-group qg. Returns the 4 normalized attnT tiles.
            Score/exp emission runs 2 pair-tiles ahead of the AV matmuls,
            crossing head boundaries, so PE never waits on exp latency."""
            chunks = cfg[qg]
            nck = len(chunks)
            at_t = [atp.tile([128, TG], BF16, tag=f"at{lh}", name=f"at{qg}_{lh}")
                    for lh in range(4)]
            if nck == 0:
                for lh in range(4):
                    nc.vector.memset(at_t[lh][:], 0.0)
                return at_t

            pairs = [chunks[i:i + 2] for i in range(0, nck, 2)]
            npairs = len(pairs)
            units = [(lh, pi) for lh in range(4) for pi in range(npairs)]
            odt, acct, pts = {}, {}, {}
            norm_pending = []      # head indices whose AV is fully emitted
            avdone = {}            # head -> unit index when its last AV was emitted
            pending = []

            def emit_av(lh, pi):
                pt, pair = pts.pop((lh, pi))
                lk = lh % 2
                for k, (c, mi, s0, m0, m1) in enumerate(pair):
                    idx = 2 * pi + k
                    nc.tensor.matmul(odt[lh][:, s0:], vres[c][:, lk * D:(lk + 1) * D],
                                     pt[:, k * TG + s0:(k + 1) * TG],
                                     start=(idx == 0), stop=(idx == nck - 1),
                                     skip_group_check=True)
                if pi == npairs - 1:
                    norm_pending.append(lh)

            def finish_normalize():
                lh = norm_pending.pop(0)
                bt = singp.tile([128, TG], F32, tag="sg", name=f"bt{qg}_{lh}")
                if rows_ok:
                    # single matmul: all-ones stationary broadcasts the
                    # partition-sum of acc to every output partition
                    nc.tensor.matmul(bt[:], ones_b[:], acct[lh][:],
                                     start=True, stop=True)
                else:
                    ddp = singp.tile([1, TG], F32, tag="sg", name=f"dd{qg}_{lh}")
                    nc.tensor.matmul(ddp[:], ones_b[:, 0:1], acct[lh][:],
                                     start=True, stop=True)
                    den = rcp.tile([1, TG], F32R, tag="den", name=f"den{qg}_{lh}")
                    nc.scalar.activation(den[:], ddp[:], COPY, bias=1e-30)
                    nc.tensor.matmul(bt[:], ones_row[:], den[:], start=True, stop=True)
                rec = rcp.tile([128, TG], F32, tag="rec", name=f"rec{qg}_{lh}")
                nc.vector.reciprocal_approx_fast(rec[:], bt[:])
                with nc.allow_low_precision(reason="bf16 attn output for wo matmul"):
                    nc.vector.tensor_mul(at_t[lh][:], odt[lh][:], rec[:])

            for ui, (lh, pi) in enumerate(units):
                if norm_pending and ui > avdone.get(norm_pending[0], 1 << 30):
                    finish_normalize()   # >=1 unit of slack after the last AV
                if pi == 0:
                    odt[lh] = singp.tile([128, TG], F32, tag="sg", name=f"od{qg}_{lh}")
                    acct[lh] = accp.tile([128, TG], BF16, tag="acc", name=f"acc{qg}_{lh}")
                lk = lh % 2
                pair = pairs[pi]
                sp = pairp.tile([128, 2 * TG], F32, tag="pp", name=f"sp{qg}_{lh}_{pi}")
                for k, (c, mi, s0, m0, m1) in enumerate(pair):
                    nc.tensor.matmul(sp[:, k * TG + s0:(k + 1) * TG],
                                     krot[lk][:, c * 128:(c + 1) * 128],
                                     qrot[lh][:, s0:], start=True, stop=True)
                pt = ptp.tile([128, 2 * TG], BF16, tag="pt", name=f"pt{qg}_{lh}_{pi}")
                lo = pair[0][2]
                hi = (len(pair) - 1) * TG + TG
                with nc.allow_low_precision(reason="bf16 softmax weights"):
                    nc.scalar.activation(pt[:, lo:hi], sp[:, lo:hi], EXP, scale=SCALE)
                for k, (c, mi, s0, m0, m1) in enumerate(pair):
                    if mi >= 0:
                        if mask_resident:
                            mtile = mt_b[:, mi * TG + m0:mi * TG + m1]
                        else:
                            mst = msp.tile([128, TG], BF16, tag="ms",
                                           name=f"ms{qg}_{lh}_{pi}")
                            nc.sync.dma_start(mst[:], mtd[:, mi * TG:(mi + 1) * TG])
                            mtile = mst[:, m0:m1]
                        nc.vector.tensor_mul(pt[:, k * TG + m0:k * TG + m1],
                                             pt[:, k * TG + m0:k * TG + m1],
                                             mtile)
                with nc.allow_low_precision(reason="bf16 denominator accumulator"):
                    for k, (c, mi, s0, m0, m1) in enumerate(pair):
                        idx = 2 * pi + k
                        if idx == 0:
                            nc.vector.tensor_copy(acct[lh][:], pt[:, 0:TG])
                        else:
                            nc.vector.tensor_add(acct[lh][:, s0:], acct[lh][:, s0:],
                                                 pt[:, k * TG + s0:(k + 1) * TG])
                pts[(lh, pi)] = (pt, pair)
                pending.append((lh, pi))
                while len(pending) > 2:
                    u = pending.pop(0)
                    emit_av(*u)
                    if u[1] == npairs - 1:
                        avdone[u[0]] = ui
            while pending:
                u = pending.pop(0)
                emit_av(*u)
            while norm_pending:
                finish_normalize()
            return at_t

        def emit_wo(qg, at_t):
            for ts in range(4):
                ob = obp.tile([128, E], BF16, tag="ob", name=f"ob{qg}_{ts}")
                for eb in range(NTG):
                    wps = singp.tile([128, TG], F32, tag="sg", name=f"w{qg}_{ts}_{eb}")
                    for lh in range(4):
                        nc.tensor.matmul(wps[:], at_t[lh][:, ts * 128:(ts + 1) * 128],
                                         wo_b[:, lh * E + eb * TG:lh * E + (eb + 1) * TG],
                                         start=(lh == 0), stop=(lh == 3))
                    if eb % 2 == 0:
                        nc.scalar.copy(ob[:, eb * TG:(eb + 1) * TG], wps[:])
                    else:
                        with nc.allow_low_precision(reason="bf16 output tile"):
                            nc.vector.tensor_copy(ob[:, eb * TG:(eb + 1) * TG], wps[:])
                nc.sync.dma_start(
                    out[qg * TG + ts * 128:qg * TG + (ts + 1) * 128, :], ob[:])

        if causal_compat:
            # schedule: proj0 attn0 proj1 wo0 attn1 proj2 wo1 attn2 proj3 wo2 attn3 wo3
            qrot = emit_proj(0, prefetch_x(0))
            if mt_b is not None:
                nc.sync.dma_start(mt_b[:], mtd[:, :])
            nc.sync.dma_start(wo_b[:], wog[:, :])
            xtb_next = prefetch_x(1)
            for tg in range(NTG):
                at_t = emit_attention(tg, qrot)
                if tg + 1 < NTG:
                    qrot = emit_proj(tg + 1, xtb_next)
                    if tg + 2 < NTG:
                        xtb_next = prefetch_x(tg + 2)
                emit_wo(tg, at_t)
        else:
            # general masks: project everything (Q^T spilled to DRAM), then attend
            if mt_b is not None:
                nc.sync.dma_start(mt_b[:], mtd[:, :])
            nc.sync.dma_start(wo_b[:], wog[:, :])
            for tg in range(NTG):
                emit_proj(tg, prefetch_x(tg))
            for qg in range(NTG):
                qrot = [atp.tile([128, TG], BF16, tag=f"qr{lh}", name=f"ql{qg}_{lh}")
                        for lh in range(4)]
                for lh in range(4):
                    nc.sync.dma_start(qrot[lh][:], qscr[qg, lh, :, :])
                at_t = emit_attention(qg, qrot)
                emit_wo(qg, at_t)

    nc.compile()
    return nc


def _host_prep(x, mask, wq, wk, wv, wo):
    m2dm = np.asarray(mask).reshape(T, T) != 0
    cfg = []
    mask_tiles = []
    mask_key = {}
    for qg in range(NTG):
        qs = slice(qg * TG, (qg + 1) * TG)
        lst = []
        for c in range(T // 128):
            sub = m2dm[qs, c * 128:(c + 1) * 128]   # [512 q, 128 kv]
            if not sub.any():
                continue
            vis_q = sub.any(1)
            full_q = sub.all(1)
            q0 = int(np.argmax(vis_q))
            s0 = q0
            nf = ~full_q
            nf[:s0] = False
            if nf.any():
                m0 = int(np.argmax(nf))
                m1 = int(len(nf) - np.argmax(nf[::-1]))
                tilea = np.ascontiguousarray(sub.T).astype(np_bf16)
                key = tilea.tobytes()
                if key not in mask_key:
                    mask_key[key] = len(mask_tiles)
                    mask_tiles.append(tilea)
                mi = mask_key[key]
            else:
                mi, m0, m1 = -1, 0, 0
            lst.append((c, mi, s0, m0, m1))
        cfg.append(tuple(lst))
    cfg = tuple(cfg)
    if mask_tiles:
        mt = np.ascontiguousarray(np.concatenate(mask_tiles, 1))   # [128, nmt*TG]
    else:
        mt = np.zeros((128, TG), np_bf16)

    inv = 1.0 / (10000.0 ** (np.arange(64, dtype=np.float64) / 64))
    ang = np.arange(T, dtype=np.float64)[:, None] * inv[None, :]
    c64 = np.cos(ang).astype(np.float32).T
    s64 = np.sin(ang).astype(np.float32).T
    c2 = np.ascontiguousarray(np.concatenate([c64, c64], 0))
    s2 = np.ascontiguousarray(np.concatenate([s64, s64], 0))
    ones = np.ones((128, 128), np.float32)
    onesb = np.ones((128, 128), np_bf16)
    rows_ok = bool(m2dm.any(1).all())

    def pmaj(a, nchunk):
        """[nchunk*128, W] -> partition-major [128, nchunk*W]."""
        w = a.shape[1]
        return np.ascontiguousarray(
            a.reshape(nchunk, 128, w).transpose(1, 0, 2).reshape(128, nchunk * w))

    in_maps = []
    for b in range(B):
        xTb = pmaj(np.ascontiguousarray(x[b].T), NEC).reshape(128, NEC, T).astype(np_bf16)
        for g in range(4):
            heads = [2 * g, 2 * g + 1, 2 * g + 8, 2 * g + 9]
            kvh = [2 * g, 2 * g + 1]
            wq_a = np.empty((E, 4 * D), np.float32)
            for lh, h in enumerate(heads):
                wq_a[:, lh * D:lh * D + 64] = wq[:, h * D:(h + 1) * D:2]
                wq_a[:, lh * D + 64:(lh + 1) * D] = wq[:, h * D + 1:(h + 1) * D:2]
            wk_a = np.empty((E, 2 * D), np.float32)
            for lk, h in enumerate(kvh):
                wk_a[:, lk * D:lk * D + 64] = wk[:, h * D:(h + 1) * D:2]
                wk_a[:, lk * D + 64:(lk + 1) * D] = wk[:, h * D + 1:(h + 1) * D:2]
            wv_g = np.ascontiguousarray(wv[:, kvh[0] * D:(kvh[0] + 2) * D])
            wo_g = np.concatenate([wo[h * D:(h + 1) * D] for h in heads], 0)
            in_maps.append({
                "xT": xTb,
                "wqa": pmaj(wq_a, NEC).astype(np_bf16),
                "wka": pmaj(wk_a, NEC).astype(np_bf16),
                "wvg": pmaj(wv_g, NEC).astype(np_bf16),
                "wog": pmaj(wo_g, 4).astype(np_bf16),
                "c2": c2, "s2": s2, "ones": ones, "onesb": onesb, "mt": mt,
            })
    return cfg, rows_ok, in_maps


def kernel(x, mask, wq, wk, wv, wo, _profile=None):
    x = np.ascontiguousarray(np.asarray(x, dtype=np.float32))
    wq = np.asarray(wq, dtype=np.float32)
    wk = np.asarray(wk, dtype=np.float32)
    wv = np.asarray(wv, dtype=np.float32)
    wo = np.asarray(wo, dtype=np.float32)
    cfg, rows_ok, in_maps = _host_prep(x, mask, wq, wk, wv, wo)
    if (cfg, rows_ok) not in _cache:
        _cache[(cfg, rows_ok)] = _build_program(cfg, rows_ok)
    nc = _cache[(cfg, rows_ok)]
    kwargs = dict(_profile) if _profile else {}
    res = run_bass_kernel_spmd(nc, in_maps, core_ids=list(range(8)), **kwargs)
    if _profile is not None and isinstance(_profile, dict):
        _profile["result"] = res
    outp = np.zeros((B, T, E), np.float32)
    for b in range(B):
        for g in range(4):
            outp[b] += res.results[b * 4 + g]["o"].astype(np.float32)
    return outp
